# revision 21
# baseline (speedup 1.0000x reference)
"""Bimamba-v2 encoder (4 layers) on 8 TRN2 NeuronCores.

Sharding: DP over batch (2) x TP over d_inner (4).
Core c: batch b = c // 4, d_inner slice q = c % 4 (channels q*256:(q+1)*256).

Per layer, per core:
  residual/RMSNorm in [d_model x token] layout (cross-partition reduce via PE)
  in_proj (fp32r matmuls) -> xi, zi slices
  depthwise causal conv fwd + anti-causal rev (DVE tensor_scalar chains)
  x_proj partials -> AllReduce over the 4 TP cores -> dt/B/C
  selective scan per (branch, d-tile, n): ACT exp (per-partition scale A),
    DVE tensor_tensor_scan (reverse branch via negative-stride APs),
  C-contraction via bf16 multiplies + DMA-accumulate tree
  y gate/skip, AllGather y -> full out_proj on every core
Final fused add + RMSNorm.
"""
import contextlib
import numpy as np

import concourse.bacc as bacc
import concourse.mybir as mybir
import concourse.tile as tile
from concourse.alu_op_type import AluOpType as ALU

F32 = mybir.dt.float32
F32R = mybir.dt.float32r
BF16 = mybir.dt.bfloat16
AF = mybir.ActivationFunctionType

NL = 4          # layers
DM = 512        # d_model
DIN = 1024      # d_inner
DC = 256        # d_inner per core (TP=4)
NST = 16        # d_state
L = 512         # seq len
EPS = 1e-5
GROUPS = [[0, 1, 2, 3], [4, 5, 6, 7]]
NCORES = 8

SIM_COMPAT = False  # CoreSim has no Silu: emulate with sigmoid+mult when True
TIMELINE_MODE = False  # replace collectives with DMA copies for TimelineSim

# dtype knobs for the scan pipeline
DBU_DT = BF16   # dBu = delta*u*B going into the scan
H_DT = BF16     # scan output
CH_DT = BF16    # C*h products + DMA-accumulate tree


def _emit_kernel(nc):
    """Emit the full 4-layer encoder program (same program on all cores)."""
    io = {}
    io["xT_d"] = nc.dram_tensor("xT", [DM, L], F32, kind="ExternalInput")
    io["normw_d"] = nc.dram_tensor("normw", [128, NL * 4], F32, kind="ExternalInput")
    io["normf_d"] = nc.dram_tensor("normf", [128, 4], F32, kind="ExternalInput")
    io["win_d"] = nc.dram_tensor("win", [NL, 4, 128, 512], F32R, kind="ExternalInput")
    io["wout_d"] = nc.dram_tensor("wout", [NL, 8, 128, 512], F32R, kind="ExternalInput")
    io["xpw_d"] = nc.dram_tensor("xpw", [NL, 128, 2 * 2 * 64], F32R, kind="ExternalInput")
    io["dtw_d"] = nc.dram_tensor("dtw", [NL, 2, 32, 256], F32, kind="ExternalInput")
    io["convw_d"] = nc.dram_tensor("convw", [128, NL * 2 * 2 * 4], F32, kind="ExternalInput")
    io["convb_d"] = nc.dram_tensor("convb", [128, NL * 2 * 2], F32, kind="ExternalInput")
    io["dtb_d"] = nc.dram_tensor("dtb", [128, NL * 2 * 2], F32, kind="ExternalInput")
    io["A_d"] = nc.dram_tensor("A", [128, NL * 2 * 2 * NST], F32, kind="ExternalInput")
    io["dsk_d"] = nc.dram_tensor("dsk", [128, NL * 2 * 2], F32, kind="ExternalInput")
    io["out_d"] = nc.dram_tensor("out_x", [4, 128, L], F32, kind="ExternalOutput")
    io["feats_d"] = nc.dram_tensor("feats", [NL, 4, 128, L], F32, kind="ExternalOutput")

    with tile.TileContext(nc) as tc:
        _body(nc, tc, io)
    nc.compile()
    return nc


def _rep_bcast(src_row_ap):
    """AP for a DMA that replicates one SBUF row across 128 partitions."""
    return src_row_ap.rearrange("p (o t) -> p o t", o=1).broadcast_to([1, 128, L])


def _body(nc, tc, io):
    ctx = contextlib.ExitStack()
    with ctx:
        pers = ctx.enter_context(tc.tile_pool(name="pers", bufs=1))
        wpool = ctx.enter_context(tc.tile_pool(name="wts", bufs=2))
        big1 = ctx.enter_context(tc.tile_pool(name="big1", bufs=1))
        act = ctx.enter_context(tc.tile_pool(name="act", bufs=1))
        act2 = ctx.enter_context(tc.tile_pool(name="act2", bufs=2))
        reps = ctx.enter_context(tc.tile_pool(name="reps", bufs=4))
        spool = ctx.enter_context(tc.tile_pool(name="scan", bufs=2))
        chp = ctx.enter_context(tc.tile_pool(name="chp", bufs=2))
        psum = ctx.enter_context(tc.tile_pool(name="psum", bufs=1, space="PSUM"))
        dram = ctx.enter_context(tc.tile_pool(name="dram", bufs=2, space="DRAM"))

        # ---- persistent smalls ----
        ones = pers.tile([128, 1], F32)
        nc.gpsimd.memset(ones[:], 1.0)
        ones_r = pers.tile([128, 1], F32R)
        nc.scalar.copy(ones_r[:], ones[:])
        epst = pers.tile([1, 1], F32)
        nc.gpsimd.memset(epst[:], EPS)
        normw = pers.tile([128, NL * 4], F32)
        nc.sync.dma_start(normw[:], io["normw_d"][:])
        normf = pers.tile([128, 4], F32)
        nc.sync.dma_start(normf[:], io["normf_d"][:])
        convw = pers.tile([128, NL, 2, 2, 4], F32)
        nc.sync.dma_start(convw[:].rearrange("p i b t k -> p (i b t k)"), io["convw_d"][:])
        convb = pers.tile([128, NL, 2, 2, 1], F32)
        nc.sync.dma_start(convb[:].rearrange("p i b t o -> p (i b t o)"), io["convb_d"][:])
        dtb = pers.tile([128, NL, 2, 2, 1], F32)
        nc.sync.dma_start(dtb[:].rearrange("p i b t o -> p (i b t o)"), io["dtb_d"][:])
        Asb = pers.tile([128, NL, 2, 2, NST], F32)
        nc.sync.dma_start(Asb[:].rearrange("p i b t n -> p (i b t n)"), io["A_d"][:])
        dsk = pers.tile([128, NL, 2, 2, 1], F32)
        nc.sync.dma_start(dsk[:].rearrange("p i b t o -> p (i b t o)"), io["dsk_d"][:])

        # residual stream + current layer output, [128 part, 4 dm-tiles, L]
        resid = pers.tile([128, 4, L], F32)
        xT_t = io["xT_d"][:].rearrange("(j p) t -> j p t", p=128)
        for j in range(4):
            nc.sync.dma_start(resid[:, j, :], xT_t[j])
        hs = pers.tile([128, 4, L], F32)
        hn = pers.tile([128, 4, L], F32R)

        def flat(ap):
            return ap.rearrange("p j t -> p (j t)")

        def rmsnorm(src, wcol, dst, scratch=None):
            """dst = src * w(per-partition) * rsqrt(mean_dm(src^2) + eps)."""
            sq = scratch if scratch is not None else dst  # consumed by PE before dst write
            nc.scalar.activation(flat(sq[:]), flat(src[:]), AF.Square)
            v = psum.tile([1, L], F32, tag="v")
            for j in range(4):
                nc.tensor.matmul(v[:], ones_r[:], sq[:, j, :],
                                 start=(j == 0), stop=(j == 3))
            # rsqrt(mean+eps) = exp(-0.5 * ln(v/DM + eps)); keeps ACT on the
            # ln/exp table (no Sqrt-table switch)
            lnv = act.tile([1, L], F32, tag="lnv")
            nc.scalar.activation(lnv[:], v[:], AF.Ln, bias=epst[:], scale=1.0 / DM)
            rstd = act.tile([1, L], F32, tag="rstd")
            nc.scalar.activation(rstd[:], lnv[:], AF.Exp, scale=-0.5)
            rstd_b = act.tile([128, L], F32, tag="rstdb")
            nc.sync.dma_start(rstd_b[:], _rep_bcast(rstd[0:1, :]))
            for j in range(4):
                nc.vector.scalar_tensor_tensor(dst[:, j, :], src[:, j, :], wcol(j),
                                               rstd_b[:], ALU.mult, ALU.mult)

        for i in range(NL):
            # ---- stream this layer's weights ----
            win = wpool.tile([128, 4, 512], F32R, tag="win", bufs=1)
            for kt in range(4):
                nc.sync.dma_start(win[:, kt, :], io["win_d"][i, kt])
            wout = wpool.tile([128, 8, 512], F32R, tag="wout", bufs=1)
            for kt in range(8):
                nc.sync.dma_start(wout[:, kt, :], io["wout_d"][i, kt])
            xpw = wpool.tile([128, 2, 2, 64], F32R, tag="xpw")
            nc.sync.dma_start(xpw[:].rearrange("p b k m -> p (b k m)"), io["xpw_d"][i])
            dtw = wpool.tile([32, 2, 256], F32, tag="dtw")
            for br in range(2):
                nc.sync.dma_start(dtw[:, br, :], io["dtw_d"][i, br])

            # ---- residual + prenorm ----
            if i > 0:
                nc.vector.tensor_tensor(flat(resid[:]), flat(resid[:]), flat(hs[:]), ALU.add)
            rmsnorm(resid, lambda j: normw[:, i * 4 + j: i * 4 + j + 1], hn)

            # ---- in_proj: xi (pt 0,1), zi (pt 2,3) ----
            xz = [psum.tile([128, L], F32, tag=f"mm{pt}", name=f"xz{pt}") for pt in range(4)]
            for pt in range(4):
                for kt in range(4):
                    nc.tensor.matmul(xz[pt][:],
                                     win[:, kt, pt * 128:(pt + 1) * 128],
                                     hn[:, kt, :],
                                     start=(kt == 0), stop=(kt == 3))
            silu_z = [act.tile([128, L], F32, tag=f"sz{t}", name=f"sz{t}") for t in range(2)]
            for t in range(2):
                if SIM_COMPAT:
                    sgz = act2.tile([128, L], F32, tag="ca", name="sgz")
                    nc.scalar.activation(sgz[:], xz[2 + t][:], AF.Sigmoid)
                    nc.vector.tensor_tensor(silu_z[t][:], sgz[:], xz[2 + t][:], ALU.mult)
                else:
                    nc.scalar.activation(silu_z[t][:], xz[2 + t][:], AF.Silu)
            xpad = [act.tile([128, L + 6], F32, tag=f"xp{t}", name=f"xp{t}") for t in range(2)]
            for t in range(2):
                nc.gpsimd.memset(xpad[t][:, 0:3], 0.0)
                nc.gpsimd.memset(xpad[t][:, L + 3:L + 6], 0.0)
                nc.scalar.copy(xpad[t][:, 3:L + 3], xz[t][:])

            # ---- conv + silu per branch (fwd causal, rev anti-causal) ----
            xc = {}
            for br in range(2):
                for t in range(2):
                    w = lambda k: convw[:, i, br, t, k:k + 1]
                    off = (lambda k: k) if br == 0 else (lambda k: 6 - k)
                    c0 = act2.tile([128, L], F32, tag="ca")
                    nc.vector.tensor_scalar(c0[:], xpad[t][:, off(0):off(0) + L], w(0), None, ALU.mult)
                    c1 = act2.tile([128, L], F32, tag="cb")
                    nc.vector.scalar_tensor_tensor(c1[:], xpad[t][:, off(1):off(1) + L], w(1), c0[:], ALU.mult, ALU.add)
                    c2 = act2.tile([128, L], F32, tag="ca")
                    nc.vector.scalar_tensor_tensor(c2[:], xpad[t][:, off(2):off(2) + L], w(2), c1[:], ALU.mult, ALU.add)
                    c3 = act2.tile([128, L], F32, tag="cb")
                    nc.vector.scalar_tensor_tensor(c3[:], xpad[t][:, off(3):off(3) + L], w(3), c2[:], ALU.mult, ALU.add)
                    xcbt = act.tile([128, L], F32R, tag=f"xc{br}{t}")
                    if SIM_COMPAT:
                        pre = act2.tile([128, L], F32, tag="ca", name="pre")
                        nc.vector.tensor_scalar(pre[:], c3[:], convb[:, i, br, t, :], None, ALU.add)
                        sgc = act2.tile([128, L], F32, tag="cb", name="sgc")
                        nc.scalar.activation(sgc[:], pre[:], AF.Sigmoid)
                        nc.vector.tensor_tensor(xcbt[:], sgc[:], pre[:], ALU.mult)
                    else:
                        nc.scalar.activation(xcbt[:], c3[:], AF.Silu, bias=convb[:, i, br, t, :])
                    xc[br, t] = xcbt

            # ---- x_proj partials + AllReduce across the TP group ----
            xdp = [psum.tile([64, L], F32, tag=f"xdbl{br}", name=f"xdp{br}")
                   for br in range(2)]
            for br in range(2):
                for kt in range(2):
                    nc.tensor.matmul(xdp[br][:],
                                     xpw[:, br, kt, :],
                                     xc[br, kt][:],
                                     start=(kt == 0), stop=(kt == 1))
            xdbl_part = act.tile([64, 2, L], F32, tag="xdblp")
            for br in range(2):
                nc.scalar.copy(xdbl_part[:, br, :], xdp[br][:])
            ar_in = dram.tile([128, L], F32, tag="arin")
            ar_out = dram.tile([128, L], F32, tag="arout")
            for br in range(2):
                nc.sync.dma_start(ar_in[br * 64:(br + 1) * 64, :], xdbl_part[:, br, :])
            if TIMELINE_MODE:
                nc.sync.dma_start(ar_out[:], ar_in[:])
            else:
                nc.gpsimd.collective_compute("AllReduce", ALU.add, ins=[ar_in.opt()],
                                             outs=[ar_out.opt()], replica_groups=GROUPS)
            xdbl = act.tile([128, L], F32, tag="xdbl")
            nc.sync.dma_start(xdbl[:], ar_out[:])

            # ---- delta = softplus(dtw @ dt_low + dtb), du = delta * xc ----
            delta, du = {}, {}
            dtlow = [act.tile([32, L], F32, tag=f"dtlow{br}", name=f"dtlow{br}")
                     for br in range(2)]
            for br in range(2):
                nc.sync.dma_start(dtlow[br][:], xdbl[br * 64:br * 64 + 32, :])
            dpre = [psum.tile([128, L], F32, tag=f"mm{k}", name=f"dpre{k}") for k in range(4)]
            for br in range(2):
                for t in range(2):
                    nc.tensor.matmul(dpre[br * 2 + t][:],
                                     dtw[:, br, t * 128:(t + 1) * 128],
                                     dtlow[br][:],
                                     start=True, stop=True)
                    # softplus(x+dtb) = ln(1 + exp(x+dtb)); x <= ~0.3 so no overflow
                    dex = act.tile([128, L], F32, tag="dex", bufs=2, name=f"dex{br}{t}")
                    nc.scalar.activation(dex[:], dpre[br * 2 + t][:], AF.Exp,
                                         bias=dtb[:, i, br, t, :])
                    dl = act.tile([128, L], F32, tag=f"dl{br}{t}")
                    nc.scalar.activation(dl[:], dex[:], AF.Ln, bias=ones[:])
                    delta[br, t] = dl
                    dut = act.tile([128, L], DBU_DT, tag=f"du{br}{t}")
                    nc.vector.tensor_tensor(dut[:], dl[:], xc[br, t][:], ALU.mult)
                    du[br, t] = dut

            # ---- B/C rows to bf16 (same lanes) ----
            bc16 = act.tile([128, L], BF16, tag="bc16")
            nc.vector.tensor_copy(bc16[32:64, :], xdbl[32:64, :])
            nc.vector.tensor_copy(bc16[96:128, :], xdbl[96:128, :])

            # ---- selective scan (n outer so B/C replicas are short-lived) ----
            y_scan = {}
            for br in range(2):
                CHs = [chp.tile([128, NST, L], CH_DT, tag="ch", name=f"ch{br}{t}")
                       for t in range(2)]
                for n in range(NST):
                    rb = reps.tile([128, L], BF16, tag="brep", name=f"rb{n}")
                    nc.sync.dma_start(rb[:], _rep_bcast(bc16[32 + br * 64 + n:33 + br * 64 + n, :]))
                    rc = reps.tile([128, L], BF16, tag="crep", name=f"rc{n}")
                    nc.scalar.dma_start(rc[:], _rep_bcast(bc16[48 + br * 64 + n:49 + br * 64 + n, :]))
                    for t in range(2):
                        dA = spool.tile([128, L], F32, tag="dA")
                        nc.scalar.activation(dA[:], delta[br, t][:], AF.Exp,
                                             scale=Asb[:, i, br, t, n:n + 1])
                        dBu = spool.tile([128, L], DBU_DT, tag="dBu")
                        nc.vector.tensor_tensor(dBu[:], du[br, t][:], rb[:], ALU.mult)
                        h = spool.tile([128, L], H_DT, tag="h")
                        if br == 0:
                            nc.vector.tensor_tensor_scan(h[:], dA[:], dBu[:], 0.0,
                                                         ALU.mult, ALU.add)
                        else:
                            nc.vector.tensor_tensor_scan(h[:, ::-1], dA[:, ::-1],
                                                         dBu[:, ::-1], 0.0,
                                                         ALU.mult, ALU.add)
                        nc.vector.tensor_tensor(CHs[t][:, n, :], h[:], rc[:], ALU.mult)
                # two parallel accumulate chains into separate tiles (an
                # accum DMA whose src and dest share a tensor faults on HW)
                for t in range(2):
                    yA = act.tile([128, L], CH_DT, tag=f"yA{br}{t}", name=f"yA{br}{t}")
                    yB = act.tile([128, L], CH_DT, tag=f"yB{br}{t}", name=f"yB{br}{t}")
                    nc.gpsimd.dma_start(yA[:], CHs[t][:, 0, :])
                    nc.gpsimd.dma_start(yB[:], CHs[t][:, 8, :])
                    for n in range(1, 8):
                        nc.gpsimd.dma_start(yA[:], CHs[t][:, n, :], accum_op=ALU.add)
                        nc.gpsimd.dma_start(yB[:], CHs[t][:, 8 + n, :], accum_op=ALU.add)
                    y_scan[br, t] = (yA, yB)

            # ---- y = (scan_f + scan_r + Df*xc_f + Dr*xc_r) * silu(z) ----
            yt = []
            for t in range(2):
                u1 = act.tile([128, L], CH_DT, tag="ua", bufs=2, name="u1")
                nc.vector.tensor_tensor(u1[:], y_scan[0, t][0][:], y_scan[0, t][1][:], ALU.add)
                u2 = act.tile([128, L], CH_DT, tag="ub", bufs=2, name="u2")
                nc.vector.tensor_tensor(u2[:], y_scan[1, t][0][:], y_scan[1, t][1][:], ALU.add)
                y1 = act.tile([128, L], F32, tag="ya", bufs=2)
                nc.vector.tensor_tensor(y1[:], u1[:], u2[:], ALU.add)
                y2 = act.tile([128, L], F32, tag="yb", bufs=2)
                nc.vector.scalar_tensor_tensor(y2[:], xc[0, t][:], dsk[:, i, 0, t, :], y1[:],
                                               ALU.mult, ALU.add)
                y3 = act.tile([128, L], F32, tag="ya", bufs=2)
                nc.vector.scalar_tensor_tensor(y3[:], xc[1, t][:], dsk[:, i, 1, t, :], y2[:],
                                               ALU.mult, ALU.add)
                y4 = act.tile([128, L], F32R, tag="yb4", bufs=2)
                nc.vector.tensor_tensor(y4[:], y3[:], silu_z[t][:], ALU.mult)
                yt.append(y4)

            # ---- AllGather y, full out_proj on every core ----
            ag_in = dram.tile([DC, L], F32R, tag="agin")
            ag_out = dram.tile([DIN, L], F32R, tag="agout")
            for t in range(2):
                nc.sync.dma_start(ag_in[t * 128:(t + 1) * 128, :], yt[t][:])
            if TIMELINE_MODE:
                nc.sync.dma_start(ag_out[0:DC, :], ag_in[:])
            else:
                nc.gpsimd.collective_compute("AllGather", ALU.bypass, ins=[ag_in.opt()],
                                             outs=[ag_out.opt()], replica_groups=GROUPS)
            yfull = big1.tile([128, 8, L], F32R, tag="yfull")
            ag_t = ag_out[:].rearrange("(k p) t -> k p t", p=128)
            for kt in range(8):
                nc.sync.dma_start(yfull[:, kt, :], ag_t[kt])
            hs_ps = [psum.tile([128, L], F32, tag=f"mm{pt}", name=f"hsps{pt}") for pt in range(4)]
            for pt in range(4):
                for kt in range(8):
                    nc.tensor.matmul(hs_ps[pt][:],
                                     wout[:, kt, pt * 128:(pt + 1) * 128],
                                     yfull[:, kt, :],
                                     start=(kt == 0), stop=(kt == 7))
            for pt in range(4):
                nc.scalar.copy(hs[:, pt, :], hs_ps[pt][:])
            for j in range(4):
                nc.sync.dma_start(io["feats_d"][i, j], hs[:, j, :])

        # ---- final add + norm (reuse hn as output buffer) ----
        nc.vector.tensor_tensor(flat(resid[:]), flat(resid[:]), flat(hs[:]), ALU.add)
        outt = big1.tile([128, 4, L], F32, tag="yfull", name="outt")
        rmsnorm(resid, lambda j: normf[:, j:j + 1], outt, scratch=hn)
        for j in range(4):
            nc.sync.dma_start(io["out_d"][j], outt[:, j, :])


_NC = None


def _get_nc():
    global _NC
    if _NC is None:
        nc = bacc.Bacc("TRN2", target_bir_lowering=False, debug=False,
                       enable_asserts=False, num_devices=NCORES)
        _NC = _emit_kernel(nc)
    return _NC


def _prep_core_inputs(inp, core):
    b, q = core // 4, core % 4
    s0, s1 = q * DC, (q + 1) * DC
    f = np.float32

    def pack_small(arrs):  # [NL][2br] arrays [DC, w] -> [128, NL*2*2*w]
        out = np.stack([np.stack([np.asarray(a).reshape(2, 128, -1) for a in lay])
                        for lay in arrs])            # [NL, 2, 2, 128, w]
        out = np.transpose(out, (3, 0, 1, 2, 4)).reshape(128, -1)
        return np.ascontiguousarray(out, f)

    win = np.stack([
        np.concatenate([inp["in_proj_w"][i, s0:s1],
                        inp["in_proj_w"][i, DIN + s0:DIN + s1]], axis=0).T.reshape(4, 128, 512)
        for i in range(NL)])
    wout = np.stack([inp["out_proj_w"][i].T.reshape(8, 128, 512) for i in range(NL)])
    xpw = np.stack([
        np.transpose(
            np.stack([inp[k][i][:, s0:s1].T.reshape(2, 128, 64) for k in ("xpw_f", "xpw_r")]),
            (2, 0, 1, 3)).reshape(128, -1)
        for i in range(NL)])
    dtw = np.stack([
        np.stack([inp[k][i][s0:s1].T for k in ("dtw_f", "dtw_r")]) for i in range(NL)])
    convw = pack_small([[inp["convw_f"][i][s0:s1], inp["convw_r"][i][s0:s1]]
                        for i in range(NL)])
    convb = pack_small([[inp["convb_f"][i][s0:s1, None], inp["convb_r"][i][s0:s1, None]]
                        for i in range(NL)])
    dtbp = pack_small([[inp["dtb_f"][i][s0:s1, None], inp["dtb_r"][i][s0:s1, None]]
                       for i in range(NL)])
    A = pack_small([[-np.exp(inp["Alog_f"][i][s0:s1]), -np.exp(inp["Alog_r"][i][s0:s1])]
                    for i in range(NL)])
    dskp = pack_small([[inp["Dskip_f"][i][s0:s1, None], inp["Dskip_r"][i][s0:s1, None]]
                       for i in range(NL)])

    return {
        "xT": np.ascontiguousarray(np.asarray(inp["x"])[b].T, f),
        "normw": np.ascontiguousarray(np.asarray(inp["norm_w"]).reshape(NL * 4, 128).T, f),
        "normf": np.ascontiguousarray(np.asarray(inp["normf_w"]).reshape(4, 128).T, f),
        "win": np.ascontiguousarray(win, f),
        "wout": np.ascontiguousarray(wout, f),
        "xpw": np.ascontiguousarray(xpw, f),
        "dtw": np.ascontiguousarray(dtw, f),
        "convw": convw, "convb": convb, "dtb": dtbp, "A": A, "dsk": dskp,
    }


def kernel(**inputs):
    inputs = {k: np.asarray(v) for k, v in inputs.items()}
    nc = _get_nc()
    in_maps = [_prep_core_inputs(inputs, c) for c in range(NCORES)]
    from concourse.bass_utils import run_bass_kernel_spmd
    res = run_bass_kernel_spmd(nc, in_maps, list(range(NCORES))).results
    out = np.stack([res[0]["out_x"].reshape(DM, L).T,
                    res[4]["out_x"].reshape(DM, L).T])
    feats = np.stack([
        np.stack([res[0]["feats"][i].reshape(DM, L).T,
                  res[4]["feats"][i].reshape(DM, L).T])
        for i in range(NL)])
    return out.astype(np.float32), feats.astype(np.float32)


# revision 22
# speedup vs baseline: 58.3623x; 58.3623x over previous
"""Bimamba-v2 encoder (4 layers) on 8 TRN2 NeuronCores.

Sharding: DP over batch (2) x TP over d_inner (4).
Core c: batch b = c // 4, d_inner slice q = c % 4 (channels q*256:(q+1)*256).

Per layer, per core:
  residual/RMSNorm in [d_model x token] layout (cross-partition reduce via PE)
  in_proj (fp32r matmuls) -> xi, zi slices
  depthwise causal conv fwd + anti-causal rev (DVE tensor_scalar chains)
  x_proj partials -> AllReduce over the 4 TP cores -> dt/B/C
  selective scan per (branch, d-tile, n): ACT exp (per-partition scale A),
    DVE tensor_tensor_scan (reverse branch via negative-stride APs),
  C-contraction via bf16 multiplies + DMA-accumulate tree
  y gate/skip, AllGather y -> full out_proj on every core
Final fused add + RMSNorm.
"""
import contextlib
import numpy as np

import concourse.bacc as bacc
import concourse.mybir as mybir
import concourse.tile as tile
from concourse.alu_op_type import AluOpType as ALU

F32 = mybir.dt.float32
F32R = mybir.dt.float32r
BF16 = mybir.dt.bfloat16
AF = mybir.ActivationFunctionType

NL = 4          # layers
DM = 512        # d_model
DIN = 1024      # d_inner
DC = 256        # d_inner per core (TP=4)
NST = 16        # d_state
L = 512         # seq len
EPS = 1e-5
GROUPS = [[0, 1, 2, 3], [4, 5, 6, 7]]
NCORES = 8

SIM_COMPAT = False  # CoreSim has no Silu: emulate with sigmoid+mult when True
TIMELINE_MODE = False  # replace collectives with DMA copies for TimelineSim

# dtype knobs for the scan pipeline
DBU_DT = BF16   # dBu = delta*u*B going into the scan
H_DT = BF16     # scan output
CH_DT = BF16    # C*h products + DMA-accumulate tree


def _emit_kernel(nc, reps=1):
    """Emit the full 4-layer encoder program (same program on all cores).

    reps > 1 replays the whole encoder serially inside one program — used
    only for timing (device time = (t_reps - t_1) / (reps - 1)).
    """
    io = {}
    io["xT_d"] = nc.dram_tensor("xT", [DM, L], F32, kind="ExternalInput")
    io["normw_d"] = nc.dram_tensor("normw", [128, NL * 4], F32, kind="ExternalInput")
    io["normf_d"] = nc.dram_tensor("normf", [128, 4], F32, kind="ExternalInput")
    io["win_d"] = nc.dram_tensor("win", [NL, 4, 128, 512], F32R, kind="ExternalInput")
    io["wout_d"] = nc.dram_tensor("wout", [NL, 8, 128, 512], F32R, kind="ExternalInput")
    io["xpw_d"] = nc.dram_tensor("xpw", [NL, 128, 2 * 2 * 64], F32R, kind="ExternalInput")
    io["dtw_d"] = nc.dram_tensor("dtw", [NL, 2, 32, 256], F32, kind="ExternalInput")
    io["convw_d"] = nc.dram_tensor("convw", [128, NL * 2 * 2 * 4], F32, kind="ExternalInput")
    io["convb_d"] = nc.dram_tensor("convb", [128, NL * 2 * 2], F32, kind="ExternalInput")
    io["dtb_d"] = nc.dram_tensor("dtb", [128, NL * 2 * 2], F32, kind="ExternalInput")
    io["A_d"] = nc.dram_tensor("A", [128, NL * 2 * 2 * NST], F32, kind="ExternalInput")
    io["dsk_d"] = nc.dram_tensor("dsk", [128, NL * 2 * 2], F32, kind="ExternalInput")
    io["out_d"] = nc.dram_tensor("out_x", [4, 128, L], F32, kind="ExternalOutput")
    io["feats_d"] = nc.dram_tensor("feats", [NL, 4, 128, L], F32, kind="ExternalOutput")

    with tile.TileContext(nc) as tc:
        for _ in range(reps):
            _body(nc, tc, io)
    nc.compile()
    return nc


def _rep_bcast(src_row_ap):
    """AP for a DMA that replicates one SBUF row across 128 partitions."""
    return src_row_ap.rearrange("p (o t) -> p o t", o=1).broadcast_to([1, 128, L])


def _body(nc, tc, io):
    ctx = contextlib.ExitStack()
    with ctx:
        pers = ctx.enter_context(tc.tile_pool(name="pers", bufs=1))
        wpool = ctx.enter_context(tc.tile_pool(name="wts", bufs=2))
        big1 = ctx.enter_context(tc.tile_pool(name="big1", bufs=1))
        act = ctx.enter_context(tc.tile_pool(name="act", bufs=1))
        act2 = ctx.enter_context(tc.tile_pool(name="act2", bufs=2))
        reps = ctx.enter_context(tc.tile_pool(name="reps", bufs=4))
        spool = ctx.enter_context(tc.tile_pool(name="scan", bufs=2))
        chp = ctx.enter_context(tc.tile_pool(name="chp", bufs=2))
        psum = ctx.enter_context(tc.tile_pool(name="psum", bufs=1, space="PSUM"))
        dram = ctx.enter_context(tc.tile_pool(name="dram", bufs=2, space="DRAM"))

        # ---- persistent smalls ----
        ones = pers.tile([128, 1], F32)
        nc.gpsimd.memset(ones[:], 1.0)
        ones_r = pers.tile([128, 1], F32R)
        nc.scalar.copy(ones_r[:], ones[:])
        epst = pers.tile([1, 1], F32)
        nc.gpsimd.memset(epst[:], EPS)
        normw = pers.tile([128, NL * 4], F32)
        nc.sync.dma_start(normw[:], io["normw_d"][:])
        normf = pers.tile([128, 4], F32)
        nc.sync.dma_start(normf[:], io["normf_d"][:])
        convw = pers.tile([128, NL, 2, 2, 4], F32)
        nc.sync.dma_start(convw[:].rearrange("p i b t k -> p (i b t k)"), io["convw_d"][:])
        convb = pers.tile([128, NL, 2, 2, 1], F32)
        nc.sync.dma_start(convb[:].rearrange("p i b t o -> p (i b t o)"), io["convb_d"][:])
        dtb = pers.tile([128, NL, 2, 2, 1], F32)
        nc.sync.dma_start(dtb[:].rearrange("p i b t o -> p (i b t o)"), io["dtb_d"][:])
        Asb = pers.tile([128, NL, 2, 2, NST], F32)
        nc.sync.dma_start(Asb[:].rearrange("p i b t n -> p (i b t n)"), io["A_d"][:])
        dsk = pers.tile([128, NL, 2, 2, 1], F32)
        nc.sync.dma_start(dsk[:].rearrange("p i b t o -> p (i b t o)"), io["dsk_d"][:])

        # residual stream + current layer output, [128 part, 4 dm-tiles, L]
        resid = pers.tile([128, 4, L], F32)
        xT_t = io["xT_d"][:].rearrange("(j p) t -> j p t", p=128)
        for j in range(4):
            nc.sync.dma_start(resid[:, j, :], xT_t[j])
        hs = pers.tile([128, 4, L], F32)
        hn = pers.tile([128, 4, L], F32R)

        def flat(ap):
            return ap.rearrange("p j t -> p (j t)")

        def rmsnorm(src, wcol, dst, scratch=None):
            """dst = src * w(per-partition) * rsqrt(mean_dm(src^2) + eps)."""
            sq = scratch if scratch is not None else dst  # consumed by PE before dst write
            nc.scalar.activation(flat(sq[:]), flat(src[:]), AF.Square)
            v = psum.tile([1, L], F32, tag="v")
            for j in range(4):
                nc.tensor.matmul(v[:], ones_r[:], sq[:, j, :],
                                 start=(j == 0), stop=(j == 3))
            # rsqrt(mean+eps) = exp(-0.5 * ln(v/DM + eps)); keeps ACT on the
            # ln/exp table (no Sqrt-table switch)
            lnv = act.tile([1, L], F32, tag="lnv")
            nc.scalar.activation(lnv[:], v[:], AF.Ln, bias=epst[:], scale=1.0 / DM)
            rstd = act.tile([1, L], F32, tag="rstd")
            nc.scalar.activation(rstd[:], lnv[:], AF.Exp, scale=-0.5)
            rstd_b = act.tile([128, L], F32, tag="rstdb")
            nc.sync.dma_start(rstd_b[:], _rep_bcast(rstd[0:1, :]))
            for j in range(4):
                nc.vector.scalar_tensor_tensor(dst[:, j, :], src[:, j, :], wcol(j),
                                               rstd_b[:], ALU.mult, ALU.mult)

        for i in range(NL):
            # ---- stream this layer's weights ----
            win = wpool.tile([128, 4, 512], F32R, tag="win", bufs=1)
            for kt in range(4):
                nc.sync.dma_start(win[:, kt, :], io["win_d"][i, kt])
            wout = wpool.tile([128, 8, 512], F32R, tag="wout", bufs=1)
            for kt in range(8):
                nc.sync.dma_start(wout[:, kt, :], io["wout_d"][i, kt])
            xpw = wpool.tile([128, 2, 2, 64], F32R, tag="xpw")
            nc.sync.dma_start(xpw[:].rearrange("p b k m -> p (b k m)"), io["xpw_d"][i])
            dtw = wpool.tile([32, 2, 256], F32, tag="dtw")
            for br in range(2):
                nc.sync.dma_start(dtw[:, br, :], io["dtw_d"][i, br])

            # ---- residual + prenorm ----
            if i > 0:
                nc.vector.tensor_tensor(flat(resid[:]), flat(resid[:]), flat(hs[:]), ALU.add)
            rmsnorm(resid, lambda j: normw[:, i * 4 + j: i * 4 + j + 1], hn)

            # ---- in_proj: xi (pt 0,1), zi (pt 2,3) ----
            xz = [psum.tile([128, L], F32, tag=f"mm{pt}", name=f"xz{pt}") for pt in range(4)]
            for pt in range(4):
                for kt in range(4):
                    nc.tensor.matmul(xz[pt][:],
                                     win[:, kt, pt * 128:(pt + 1) * 128],
                                     hn[:, kt, :],
                                     start=(kt == 0), stop=(kt == 3))
            silu_z = [act.tile([128, L], F32, tag=f"sz{t}", name=f"sz{t}") for t in range(2)]
            for t in range(2):
                if SIM_COMPAT:
                    sgz = act2.tile([128, L], F32, tag="ca", name="sgz")
                    nc.scalar.activation(sgz[:], xz[2 + t][:], AF.Sigmoid)
                    nc.vector.tensor_tensor(silu_z[t][:], sgz[:], xz[2 + t][:], ALU.mult)
                else:
                    nc.scalar.activation(silu_z[t][:], xz[2 + t][:], AF.Silu)
            xpad = [act.tile([128, L + 6], F32, tag=f"xp{t}", name=f"xp{t}") for t in range(2)]
            for t in range(2):
                nc.gpsimd.memset(xpad[t][:, 0:3], 0.0)
                nc.gpsimd.memset(xpad[t][:, L + 3:L + 6], 0.0)
                nc.scalar.copy(xpad[t][:, 3:L + 3], xz[t][:])

            # ---- conv + silu per branch (fwd causal, rev anti-causal) ----
            xc = {}
            for br in range(2):
                for t in range(2):
                    w = lambda k: convw[:, i, br, t, k:k + 1]
                    off = (lambda k: k) if br == 0 else (lambda k: 6 - k)
                    c0 = act2.tile([128, L], F32, tag="ca")
                    nc.vector.tensor_scalar(c0[:], xpad[t][:, off(0):off(0) + L], w(0), None, ALU.mult)
                    c1 = act2.tile([128, L], F32, tag="cb")
                    nc.vector.scalar_tensor_tensor(c1[:], xpad[t][:, off(1):off(1) + L], w(1), c0[:], ALU.mult, ALU.add)
                    c2 = act2.tile([128, L], F32, tag="ca")
                    nc.vector.scalar_tensor_tensor(c2[:], xpad[t][:, off(2):off(2) + L], w(2), c1[:], ALU.mult, ALU.add)
                    c3 = act2.tile([128, L], F32, tag="cb")
                    nc.vector.scalar_tensor_tensor(c3[:], xpad[t][:, off(3):off(3) + L], w(3), c2[:], ALU.mult, ALU.add)
                    xcbt = act.tile([128, L], F32R, tag=f"xc{br}{t}")
                    if SIM_COMPAT:
                        pre = act2.tile([128, L], F32, tag="ca", name="pre")
                        nc.vector.tensor_scalar(pre[:], c3[:], convb[:, i, br, t, :], None, ALU.add)
                        sgc = act2.tile([128, L], F32, tag="cb", name="sgc")
                        nc.scalar.activation(sgc[:], pre[:], AF.Sigmoid)
                        nc.vector.tensor_tensor(xcbt[:], sgc[:], pre[:], ALU.mult)
                    else:
                        nc.scalar.activation(xcbt[:], c3[:], AF.Silu, bias=convb[:, i, br, t, :])
                    xc[br, t] = xcbt

            # ---- x_proj partials + AllReduce across the TP group ----
            xdp = [psum.tile([64, L], F32, tag=f"xdbl{br}", name=f"xdp{br}")
                   for br in range(2)]
            for br in range(2):
                for kt in range(2):
                    nc.tensor.matmul(xdp[br][:],
                                     xpw[:, br, kt, :],
                                     xc[br, kt][:],
                                     start=(kt == 0), stop=(kt == 1))
            xdbl_part = act.tile([64, 2, L], F32, tag="xdblp")
            for br in range(2):
                nc.scalar.copy(xdbl_part[:, br, :], xdp[br][:])
            ar_in = dram.tile([128, L], F32, tag="arin")
            ar_out = dram.tile([128, L], F32, tag="arout")
            for br in range(2):
                nc.sync.dma_start(ar_in[br * 64:(br + 1) * 64, :], xdbl_part[:, br, :])
            if TIMELINE_MODE:
                nc.sync.dma_start(ar_out[:], ar_in[:])
            else:
                nc.gpsimd.collective_compute("AllReduce", ALU.add, ins=[ar_in.opt()],
                                             outs=[ar_out.opt()], replica_groups=GROUPS)
            xdbl = act.tile([128, L], F32, tag="xdbl")
            nc.sync.dma_start(xdbl[:], ar_out[:])

            # ---- delta = softplus(dtw @ dt_low + dtb), du = delta * xc ----
            delta, du = {}, {}
            dtlow = [act.tile([32, L], F32, tag=f"dtlow{br}", name=f"dtlow{br}")
                     for br in range(2)]
            for br in range(2):
                nc.sync.dma_start(dtlow[br][:], xdbl[br * 64:br * 64 + 32, :])
            dpre = [psum.tile([128, L], F32, tag=f"mm{k}", name=f"dpre{k}") for k in range(4)]
            for br in range(2):
                for t in range(2):
                    nc.tensor.matmul(dpre[br * 2 + t][:],
                                     dtw[:, br, t * 128:(t + 1) * 128],
                                     dtlow[br][:],
                                     start=True, stop=True)
                    # softplus(x+dtb) = ln(1 + exp(x+dtb)); x <= ~0.3 so no overflow
                    dex = act.tile([128, L], F32, tag="dex", bufs=2, name=f"dex{br}{t}")
                    nc.scalar.activation(dex[:], dpre[br * 2 + t][:], AF.Exp,
                                         bias=dtb[:, i, br, t, :])
                    dl = act.tile([128, L], F32, tag=f"dl{br}{t}")
                    nc.scalar.activation(dl[:], dex[:], AF.Ln, bias=ones[:])
                    delta[br, t] = dl
                    dut = act.tile([128, L], DBU_DT, tag=f"du{br}{t}")
                    nc.vector.tensor_tensor(dut[:], dl[:], xc[br, t][:], ALU.mult)
                    du[br, t] = dut

            # ---- B/C rows to bf16 (same lanes) ----
            bc16 = act.tile([128, L], BF16, tag="bc16")
            nc.vector.tensor_copy(bc16[32:64, :], xdbl[32:64, :])
            nc.vector.tensor_copy(bc16[96:128, :], xdbl[96:128, :])

            # ---- selective scan (n outer so B/C replicas are short-lived) ----
            y_scan = {}
            for br in range(2):
                CHs = [chp.tile([128, NST, L], CH_DT, tag="ch", name=f"ch{br}{t}")
                       for t in range(2)]
                for n in range(NST):
                    rb = reps.tile([128, L], BF16, tag="brep", name=f"rb{n}")
                    nc.sync.dma_start(rb[:], _rep_bcast(bc16[32 + br * 64 + n:33 + br * 64 + n, :]))
                    rc = reps.tile([128, L], BF16, tag="crep", name=f"rc{n}")
                    nc.scalar.dma_start(rc[:], _rep_bcast(bc16[48 + br * 64 + n:49 + br * 64 + n, :]))
                    for t in range(2):
                        dA = spool.tile([128, L], F32, tag="dA")
                        nc.scalar.activation(dA[:], delta[br, t][:], AF.Exp,
                                             scale=Asb[:, i, br, t, n:n + 1])
                        dBu = spool.tile([128, L], DBU_DT, tag="dBu")
                        nc.vector.tensor_tensor(dBu[:], du[br, t][:], rb[:], ALU.mult)
                        h = spool.tile([128, L], H_DT, tag="h")
                        if br == 0:
                            nc.vector.tensor_tensor_scan(h[:], dA[:], dBu[:], 0.0,
                                                         ALU.mult, ALU.add)
                        else:
                            nc.vector.tensor_tensor_scan(h[:, ::-1], dA[:, ::-1],
                                                         dBu[:, ::-1], 0.0,
                                                         ALU.mult, ALU.add)
                        nc.vector.tensor_tensor(CHs[t][:, n, :], h[:], rc[:], ALU.mult)
                # two parallel accumulate chains into separate tiles (an
                # accum DMA whose src and dest share a tensor faults on HW)
                for t in range(2):
                    yA = act.tile([128, L], CH_DT, tag=f"yA{br}{t}", name=f"yA{br}{t}")
                    yB = act.tile([128, L], CH_DT, tag=f"yB{br}{t}", name=f"yB{br}{t}")
                    nc.gpsimd.dma_start(yA[:], CHs[t][:, 0, :])
                    nc.gpsimd.dma_start(yB[:], CHs[t][:, 8, :])
                    for n in range(1, 8):
                        nc.gpsimd.dma_start(yA[:], CHs[t][:, n, :], accum_op=ALU.add)
                        nc.gpsimd.dma_start(yB[:], CHs[t][:, 8 + n, :], accum_op=ALU.add)
                    y_scan[br, t] = (yA, yB)

            # ---- y = (scan_f + scan_r + Df*xc_f + Dr*xc_r) * silu(z) ----
            yt = []
            for t in range(2):
                u1 = act.tile([128, L], CH_DT, tag="ua", bufs=2, name="u1")
                nc.vector.tensor_tensor(u1[:], y_scan[0, t][0][:], y_scan[0, t][1][:], ALU.add)
                u2 = act.tile([128, L], CH_DT, tag="ub", bufs=2, name="u2")
                nc.vector.tensor_tensor(u2[:], y_scan[1, t][0][:], y_scan[1, t][1][:], ALU.add)
                y1 = act.tile([128, L], F32, tag="ya", bufs=2)
                nc.vector.tensor_tensor(y1[:], u1[:], u2[:], ALU.add)
                y2 = act.tile([128, L], F32, tag="yb", bufs=2)
                nc.vector.scalar_tensor_tensor(y2[:], xc[0, t][:], dsk[:, i, 0, t, :], y1[:],
                                               ALU.mult, ALU.add)
                y3 = act.tile([128, L], F32, tag="ya", bufs=2)
                nc.vector.scalar_tensor_tensor(y3[:], xc[1, t][:], dsk[:, i, 1, t, :], y2[:],
                                               ALU.mult, ALU.add)
                y4 = act.tile([128, L], F32R, tag="yb4", bufs=2)
                nc.vector.tensor_tensor(y4[:], y3[:], silu_z[t][:], ALU.mult)
                yt.append(y4)

            # ---- AllGather y, full out_proj on every core ----
            ag_in = dram.tile([DC, L], F32R, tag="agin")
            ag_out = dram.tile([DIN, L], F32R, tag="agout")
            for t in range(2):
                nc.sync.dma_start(ag_in[t * 128:(t + 1) * 128, :], yt[t][:])
            if TIMELINE_MODE:
                nc.sync.dma_start(ag_out[0:DC, :], ag_in[:])
            else:
                nc.gpsimd.collective_compute("AllGather", ALU.bypass, ins=[ag_in.opt()],
                                             outs=[ag_out.opt()], replica_groups=GROUPS)
            yfull = big1.tile([128, 8, L], F32R, tag="yfull")
            ag_t = ag_out[:].rearrange("(k p) t -> k p t", p=128)
            for kt in range(8):
                nc.sync.dma_start(yfull[:, kt, :], ag_t[kt])
            hs_ps = [psum.tile([128, L], F32, tag=f"mm{pt}", name=f"hsps{pt}") for pt in range(4)]
            for pt in range(4):
                for kt in range(8):
                    nc.tensor.matmul(hs_ps[pt][:],
                                     wout[:, kt, pt * 128:(pt + 1) * 128],
                                     yfull[:, kt, :],
                                     start=(kt == 0), stop=(kt == 7))
            for pt in range(4):
                nc.scalar.copy(hs[:, pt, :], hs_ps[pt][:])
            for j in range(4):
                nc.sync.dma_start(io["feats_d"][i, j], hs[:, j, :])

        # ---- final add + norm (reuse hn as output buffer) ----
        nc.vector.tensor_tensor(flat(resid[:]), flat(resid[:]), flat(hs[:]), ALU.add)
        outt = big1.tile([128, 4, L], F32, tag="yfull", name="outt")
        rmsnorm(resid, lambda j: normf[:, j:j + 1], outt, scratch=hn)
        for j in range(4):
            nc.sync.dma_start(io["out_d"][j], outt[:, j, :])


_NC = {}


def _get_nc(reps=1):
    if reps not in _NC:
        nc = bacc.Bacc("TRN2", target_bir_lowering=False, debug=False,
                       enable_asserts=False, num_devices=NCORES)
        _NC[reps] = _emit_kernel(nc, reps)
    return _NC[reps]


def _prep_core_inputs(inp, core):
    b, q = core // 4, core % 4
    s0, s1 = q * DC, (q + 1) * DC
    f = np.float32

    def pack_small(arrs):  # [NL][2br] arrays [DC, w] -> [128, NL*2*2*w]
        out = np.stack([np.stack([np.asarray(a).reshape(2, 128, -1) for a in lay])
                        for lay in arrs])            # [NL, 2, 2, 128, w]
        out = np.transpose(out, (3, 0, 1, 2, 4)).reshape(128, -1)
        return np.ascontiguousarray(out, f)

    win = np.stack([
        np.concatenate([inp["in_proj_w"][i, s0:s1],
                        inp["in_proj_w"][i, DIN + s0:DIN + s1]], axis=0).T.reshape(4, 128, 512)
        for i in range(NL)])
    wout = np.stack([inp["out_proj_w"][i].T.reshape(8, 128, 512) for i in range(NL)])
    xpw = np.stack([
        np.transpose(
            np.stack([inp[k][i][:, s0:s1].T.reshape(2, 128, 64) for k in ("xpw_f", "xpw_r")]),
            (2, 0, 1, 3)).reshape(128, -1)
        for i in range(NL)])
    dtw = np.stack([
        np.stack([inp[k][i][s0:s1].T for k in ("dtw_f", "dtw_r")]) for i in range(NL)])
    convw = pack_small([[inp["convw_f"][i][s0:s1], inp["convw_r"][i][s0:s1]]
                        for i in range(NL)])
    convb = pack_small([[inp["convb_f"][i][s0:s1, None], inp["convb_r"][i][s0:s1, None]]
                        for i in range(NL)])
    dtbp = pack_small([[inp["dtb_f"][i][s0:s1, None], inp["dtb_r"][i][s0:s1, None]]
                       for i in range(NL)])
    A = pack_small([[-np.exp(inp["Alog_f"][i][s0:s1]), -np.exp(inp["Alog_r"][i][s0:s1])]
                    for i in range(NL)])
    dskp = pack_small([[inp["Dskip_f"][i][s0:s1, None], inp["Dskip_r"][i][s0:s1, None]]
                       for i in range(NL)])

    return {
        "xT": np.ascontiguousarray(np.asarray(inp["x"])[b].T, f),
        "normw": np.ascontiguousarray(np.asarray(inp["norm_w"]).reshape(NL * 4, 128).T, f),
        "normf": np.ascontiguousarray(np.asarray(inp["normf_w"]).reshape(4, 128).T, f),
        "win": np.ascontiguousarray(win, f),
        "wout": np.ascontiguousarray(wout, f),
        "xpw": np.ascontiguousarray(xpw, f),
        "dtw": np.ascontiguousarray(dtw, f),
        "convw": convw, "convb": convb, "dtb": dtbp, "A": A, "dsk": dskp,
    }


def kernel(**inputs):
    inputs = {k: np.asarray(v) for k, v in inputs.items()}
    nc = _get_nc()
    in_maps = [_prep_core_inputs(inputs, c) for c in range(NCORES)]
    from concourse.bass_utils import run_bass_kernel_spmd
    res = run_bass_kernel_spmd(nc, in_maps, list(range(NCORES))).results
    out = np.stack([res[0]["out_x"].reshape(DM, L).T,
                    res[4]["out_x"].reshape(DM, L).T])
    feats = np.stack([
        np.stack([res[0]["feats"][i].reshape(DM, L).T,
                  res[4]["feats"][i].reshape(DM, L).T])
        for i in range(NL)])
    return out.astype(np.float32), feats.astype(np.float32)
